# revision 12
# baseline (speedup 1.0000x reference)
"""Trainium2 Bass kernel for the 4-kernel MMD permutation test (nn_DUAL_78237124264373).

Sharding: 8 cores = 2 kernel-pairs x 4 permutation quarters. Core c<4 computes
kernels (0,1) for perms [50*(c%4), 50*(c%4)+50); core c>=4 the same for
kernels (2,3). Each core evaluates TWO kernel matrices, LAPLACIAN FIRST:
the SQRT activation table is resident from the start (warm op), dist is
computed straight out of the d2 PSUM tiles with the per-row-tile sq column
fused as the activation bias, and there is a single table swap to EXP that
serves both the laplacian and the gaussian passes. Order across the swap is
pinned with zero-valued bias/scale tokens that data-depend on the previous
block's last op (baseline trick), so the Tile scheduler cannot interleave.

Per-core pipeline (slot a = laplacian, slot b = gaussian):
  d2 = L^T R on the PE in f32r (L = [Zt; 1], R = [-2 Zt; sq + B]), 12 PSUM
  pieces (6 row tiles x 512+256) in a 4-deep PSUM pool, inputs spread over
  all five DMA queue rings so the first matmul is DMA-gated at ~9us instead
  of ~11us. Scalar: sqrt(ps + sq) -> dist per tile trailing the matmuls,
  then ONE swap to EXP; slot-a K = exp(lb*dist) chunk-by-chunk with its
  M0 = A_aug K matmuls right behind, then slot-b K = exp(ga*d2sb) the same
  way (d2sb staged by the DVE from PSUM during the d2 phase). Each slot's
  row stats (aKa, aK1, colA), and the PE transpose into the partition-0
  assembly row run as soon as its M0 completes; slot-b's aK1 rowsum rides
  the idle Pool engine in parallel with the DVE. The final affine assembly
  (U, ck, U_b = ubv + ck) happens on the host from the DMA'd 512-wide
  stats row - it is a per-core scalar epilogue, part of the gather.
"""

import sys

import numpy as np

if "/opt/trn_rl_repo" not in sys.path:
    sys.path.insert(0, "/opt/trn_rl_repo")

import ml_dtypes

import concourse.bacc as bacc
import concourse.bass as bass
import concourse.mybir as mybir
import concourse.tile as tile
from concourse import bass_utils

N = 384
NM = 768
D = 64
NPER = 200
NC = 8
PPC = 50                      # perms per core
ROWS = PPC + 2                # + X-identity + Y-identity rows
NBLK = 3 * PPC + 3            # pair blocks of 128: 50 perms x 3 + stripe x 3
BIAS = 4e-3                   # keeps d2 > 0 under f32r rounding (sqrt input)
C1 = float(N * (N - 1))
C2 = float(N * N)
KAP = np.float32(2.0 / C1 + 2.0 / C2)
CB1 = np.float32(1.0 / C1 + 2.0 / C2)
CB2 = np.float32(1.0 / C1)
TCO = np.float32(2.0 / C2)
IC1 = np.float32(1.0 / C1)
IC2 = np.float32(1.0 / C2)

F32 = mybir.dt.float32
F32R = mybir.dt.float32r
BF16 = mybir.dt.bfloat16
AF = mybir.ActivationFunctionType
ALU = mybir.AluOpType

BFP_W = 6 * D + 3 * D + NM           # atp | wct | astk (need-ordered)
FSP_W = 96 + 2 * NBLK                # consts | prd2 | prds


def _build():
    nc = bacc.Bacc("TRN2", target_bir_lowering=False, debug=False)
    with tile.TileContext(nc) as tc:
        with tc.tile_pool(name="dram", bufs=1, space="DRAM") as dram, \
             tc.tile_pool(name="io", bufs=1) as io, \
             tc.tile_pool(name="big", bufs=1) as big, \
             tc.tile_pool(name="scr", bufs=1) as scr, \
             tc.tile_pool(name="sml", bufs=1) as sml:

            def din(name, shape, dt=F32):
                return dram.tile(shape, dt, kind="ExternalInput", name=name,
                                 uniquify=False)

            zlr_d = din("zlr", [D + 1, 2 * NM], F32R)
            bfp_d = din("bfp", [128, BFP_W], BF16)
            fsp_d = din("fsp", [128, FSP_W], F32)
            out_d = dram.tile([1, 512], F32, kind="ExternalOutput",
                              name="out", uniquify=False)

            # ---- input DMAs spread over the 3 DMA-capable queue rings
            # (~45GB/s each); first-needed pieces lead their ring ----
            fsp = io.tile([128, FSP_W], F32, name="fsp_sb")
            zlr = io.tile([D + 1, 2 * NM], F32R, name="zlr_sb")
            bfp = io.tile([128, BFP_W], BF16, name="bfp_sb")
            nc.scalar.dma_start(out=zlr[:, 512:640], in_=zlr_d[:, 512:640])
            nc.sync.dma_start(out=zlr[:, 0:256], in_=zlr_d[:, 0:256])
            nc.gpsimd.dma_start(out=zlr[:, 256:512], in_=zlr_d[:, 256:512])
            nc.scalar.dma_start(out=fsp[:, 0:96], in_=fsp_d[:, 0:96])
            nc.gpsimd.dma_start(out=zlr[:, 640:896], in_=zlr_d[:, 640:896])
            nc.sync.dma_start(out=zlr[:, 896:1216], in_=zlr_d[:, 896:1216])
            nc.scalar.dma_start(out=zlr[:, 1216:1536],
                                in_=zlr_d[:, 1216:1536])
            nc.sync.dma_start(out=bfp[:, 0:9 * D], in_=bfp_d[:, 0:9 * D])
            nc.gpsimd.dma_start(out=bfp[:, 9 * D:BFP_W],
                                in_=bfp_d[:, 9 * D:BFP_W])
            nc.scalar.dma_start(out=fsp[:, 96:FSP_W], in_=fsp_d[:, 96:FSP_W])

            zr1 = zlr[:, 0:512]
            zl0 = zlr[:, 512:640]
            zr2 = zlr[:, 640:896]
            zlrest = zlr[:, 896:2 * NM]
            idm = fsp[:, 32:96]
            atp = bfp[:, 0:6 * D]                    # A^T chunks, 64-padded
            wct = bfp[:, 6 * D:9 * D]
            astk = bfp[:, 9 * D:9 * D + NM]          # A rows at 0-51 / 64-115
            prd2 = fsp[:, 96:96 + NBLK]              # pair d2 (host-reduced)
            prds = fsp[:, 96 + NBLK:FSP_W]           # pair dist
            sqc = fsp[:, 0:6]                        # sq columns per row tile
            ga = fsp[:, 12:13]
            lb = fsp[:, 13:14]
            zero = fsp[:, 14:15]

            ones = io.tile([128, 1], F32, name="ones_sb")
            nc.vector.memset(ones[:], 1.0)
            onesb = io.tile([128, 1], BF16, name="onesb_sb")
            nc.vector.memset(onesb[:], 1.0)

            d2sb = big.tile([128, 6 * NM], F32, name="d2sb")
            dist = big.tile([128, 6 * NM], F32, name="dist_sb")
            kta = big.tile([128, 6 * NM], BF16, name="kta")
            ktb = big.tile([128, 6 * NM], BF16, name="ktb")
            M0sb = big.tile([128, NM], F32, name="M0sb")
            sB = scr.tile([128, NM], F32, name="sB")
            pack = sml.tile([128, 4], F32, name="pack")
            frow = sml.tile([1, 512], F32, name="frow")

            # warm the SQRT activation table while DMAs are in flight
            warm = sml.tile([128, 1], F32, name="warm")
            nc.scalar.activation(warm[0:1, :], ones[0:1, :], AF.Sqrt,
                                 bias=0.0, scale=1.0)

            # ---- d2 phase: f32r matmuls, one [128,768] PSUM tile per row
            # tile, 4 deep; sqrt(ps + sq) straight from PSUM -> dist;
            # DVE lands clamped d2 in SBUF for the gaussian exp ----
            with tc.tile_pool(name="psA", bufs=4, space="PSUM") as psA:
                for r in range(6):
                    lhs = (zl0 if r == 0 else
                           zlrest[:, 128 * (r - 1):128 * r])
                    ps_d2 = psA.tile([128, NM], F32, tag="d2",
                                     name=f"ps_d2_{r}")
                    nc.tensor.matmul(ps_d2[:, 0:512], lhs, zr1[:],
                                     start=True, stop=True)
                    nc.tensor.matmul(ps_d2[:, 512:NM], lhs, zr2[:],
                                     start=True, stop=True)
                    sl = slice(NM * r, NM * (r + 1))
                    nc.vector.tensor_scalar(
                        out=d2sb[:, sl], in0=ps_d2[:],
                        scalar1=sqc[:, r:r + 1], scalar2=0.0,
                        op0=ALU.add, op1=ALU.max)
                    # sqrt off the clamped staging tile: the PSUM value can
                    # round below -sqc on the diagonal (d2_true = BIAS)
                    nc.scalar.activation(dist[:, sl], d2sb[:, sl], AF.Sqrt,
                                         scale=1.0, bias=0.0)

            with tc.tile_pool(name="psB", bufs=1, space="PSUM") as psB, \
                 tc.tile_pool(name="psC", bufs=1, space="PSUM") as psC:

                ps_m = psB.tile([128, NM], F32, name="ps_m")
                ps_tc = psC.tile([128, 2], F32, name="ps_tc")
                ps_t = ps_tc[:, 0:1]
                ps_corr = ps_tc[:, 1:2]
                ps_row = ps_m[0:1, 0:512]

                # lbt == lb, but depends on the last column of EVERY sqrt
                # tile, so no SQRT op can be scheduled after the EXP swap
                dl0 = dist[:, NM - 1:NM]
                dlast = bass.AP(dl0.tensor, dl0.offset, [dl0.ap[0], [NM, 6]])
                zt6 = sml.tile([128, 1], F32, name="zt6")
                nc.vector.tensor_reduce(zt6[:], dlast,
                                        axis=mybir.AxisListType.X,
                                        op=ALU.max)
                lbt = sml.tile([128, 1], F32, name="lbt")
                nc.gpsimd.tensor_scalar(
                    out=lbt[:], in0=zt6[:], scalar1=0.0,
                    scalar2=lb, op0=ALU.mult, op1=ALU.add)

                def slot_tail(i, pe):
                    """Pair sums, corrections, row stats and the PE
                    transpose for slot i; runs as soon as its M0 stops."""
                    pt = slice(64 * i, 64 * i + 64)
                    # t3: per-perm 3-block partial sums; group PPC holds the
                    # stripe so t[50] = sum(e) lands in ps_t for free
                    pe3 = pe.rearrange("p (g t) -> p g t", t=3)
                    t3 = sml.tile([128, 64], BF16, name=f"t3_{i}")
                    nc.vector.memset(t3[:, PPC + 1:64], 0.0)
                    with nc.allow_low_precision(reason="3-wide bf16 sum"):
                        nc.vector.tensor_reduce(t3[:, 0:PPC + 1], pe3[:],
                                                axis=mybir.AxisListType.X,
                                                op=ALU.add)
                    nc.tensor.matmul(ps_t[pt, :], t3[:], onesb[:],
                                     start=True, stop=True,
                                     tile_position=(0, 64 * i),
                                     skip_group_check=True)
                    for c in range(3):
                        nc.tensor.matmul(
                            ps_corr[pt, :], wct[:, D * c:D * c + 64],
                            pe[:, 3 * PPC + c:3 * PPC + c + 1],
                            start=(c == 0), stop=(c == 2),
                            tile_position=(0, 64 * i),
                            skip_group_check=True)
                    # row stats off this slot's half of ps_m; slot b's aK1
                    # rowsum rides the idle Pool engine in parallel
                    nc.vector.scalar_tensor_tensor(
                        out=sB[pt, :], in0=ps_m[pt, :], scalar=1.0,
                        in1=astk[pt, :], op0=ALU.mult, op1=ALU.mult,
                        accum_out=pack[pt, 1:2])
                    if i == 1:
                        nc.scalar.activation(M0sb[pt, :], ps_m[pt, :],
                                             AF.Copy, bias=0.0, scale=1.0,
                                             accum_out=pack[pt, 2:3])
                    else:
                        nc.vector.tensor_scalar(
                            out=M0sb[pt, :], in0=ps_m[pt, :], scalar1=1.0,
                            scalar2=0.0, op0=ALU.mult, op1=ALU.add,
                            accum_out=pack[pt, 2:3])
                    # ubv = KAP*(q0 - arow) + corr + TCO*t into pack col 0;
                    # q0 / arow / t stay in cols 1-3 for the transpose
                    nc.vector.tensor_tensor(out=pack[pt, 0:1],
                                            in0=pack[pt, 1:2],
                                            in1=pack[pt, 2:3],
                                            op=ALU.subtract)
                    nc.vector.scalar_tensor_tensor(
                        out=pack[pt, 0:1], in0=pack[pt, 0:1],
                        scalar=float(KAP), in1=ps_corr[pt, :],
                        op0=ALU.mult, op1=ALU.add)
                    nc.vector.scalar_tensor_tensor(
                        out=pack[pt, 0:1], in0=ps_t[pt, :],
                        scalar=float(TCO), in1=pack[pt, 0:1],
                        op0=ALU.mult, op1=ALU.add)
                    nc.vector.tensor_copy(pack[pt, 3:4], ps_t[pt, :])
                    # transpose the 4 pack columns into the partition-0 row
                    # (ps_m bank 0 is free again: stats above read it first)
                    for k in range(4):
                        nc.tensor.matmul(
                            ps_row[0:1,
                                   128 * k + 64 * i:128 * k + 64 * i + 64],
                            pack[pt, k:k + 1], idm[pt, :],
                            is_transpose=True, start=True, stop=True,
                            tile_position=(64 * i, 0),
                            skip_group_check=True)
                    s0 = ps_row[0:1, 64 * i:64 * i + 1]
                    f0 = frow[0:1, 64 * i:64 * i + 1]
                    nc.vector.tensor_copy(
                        bass.AP(f0.tensor, f0.offset,
                                [f0.ap[0], [128, 4], [1, 64]]),
                        bass.AP(s0.tensor, s0.offset,
                                [s0.ap[0], [128, 4], [1, 64]]))

                # ---- slot a (laplacian): one swap to EXP, K chunks with
                # M0 right behind; pair exp after the first chunk ----
                pel = sml.tile([128, NBLK], BF16, name="pel")
                for c in range(3):
                    cs = slice(2 * NM * c, 2 * NM * (c + 1))
                    nc.scalar.activation(ktb[:, cs], dist[:, cs], AF.Exp,
                                         scale=lbt, bias=zero)
                    if c == 0:
                        nc.scalar.activation(pel[:], prds[:], AF.Exp,
                                             bias=zero, scale=lbt)
                    for r in (2 * c, 2 * c + 1):
                        for fs in (slice(0, 512), slice(512, NM)):
                            nc.tensor.matmul(ps_m[0:64, fs],
                                             atp[:, D * r:D * r + 64],
                                             ktb[:, NM * r + fs.start:
                                                  NM * r + fs.stop],
                                             start=(r == 0), stop=(r == 5),
                                             tile_position=(0, 0),
                                             skip_group_check=True)
                slot_tail(0, pel)

                # zbg: zero bias that depends on the last slot-a EXP op so
                # the gaussian block cannot be scheduled before it
                zbg1 = sml.tile([128, 1], F32, name="zbg1")
                nc.gpsimd.tensor_scalar(
                    out=zbg1[:], in0=ktb[:, 6 * NM - 1:6 * NM],
                    scalar1=0.0, scalar2=0.0, op0=ALU.mult, op1=ALU.add)
                zbg = sml.tile([128, 1], F32, name="zbg")
                nc.gpsimd.tensor_tensor(out=zbg[:], in0=zbg1[:],
                                        in1=pel[:, 0:1], op=ALU.mult)
                # zbg-gated copy of atp pins the slot-b M0 ordering on the
                # in-order PE queue behind the slot-a block
                atp2 = scr.tile([128, 6 * D], BF16, name="atp2")
                nc.gpsimd.tensor_scalar(
                    out=atp2[:], in0=atp[:], scalar1=1.0, scalar2=zbg[:],
                    op0=ALU.mult, op1=ALU.add)

                # ---- slot b (gaussian): exp(ga*d2sb) chunks + pair exp ----
                peg = sml.tile([128, NBLK], BF16, name="peg")
                for c in range(3):
                    cs = slice(2 * NM * c, 2 * NM * (c + 1))
                    nc.scalar.activation(kta[:, cs], d2sb[:, cs], AF.Exp,
                                         scale=ga, bias=zbg)
                    if c == 0:
                        nc.scalar.activation(peg[:], prd2[:], AF.Exp,
                                             bias=zbg, scale=ga)
                    for r in (2 * c, 2 * c + 1):
                        for fs in (slice(0, 512), slice(512, NM)):
                            nc.tensor.matmul(ps_m[64:128, fs],
                                             atp2[:, D * r:D * r + 64],
                                             kta[:, NM * r + fs.start:
                                                  NM * r + fs.stop],
                                             start=(r == 0), stop=(r == 5),
                                             tile_position=(0, 64),
                                             skip_group_check=True)
                slot_tail(1, peg)

                # raw stats row out; the affine U/ck/U_b assembly is host-side
                nc.sync.dma_start(out=out_d[:, :], in_=frow[0:1, :])

    nc.compile()
    return nc


def _host_prep(X, Y, bandwidths, perms):
    X = np.ascontiguousarray(X, np.float32)
    Y = np.ascontiguousarray(Y, np.float32)
    perms = np.ascontiguousarray(perms, np.int32)
    Zf = np.concatenate([X, Y], 0)
    Zt = Zf.T.astype(np.float32)
    sq = (Zf.astype(np.float64) ** 2).sum(1).astype(np.float32)
    b = np.asarray(bandwidths, np.float64)

    zlr = np.zeros((D + 1, 2 * NM), np.float32)
    R = np.concatenate([-2.0 * Zt, (sq + BIAS)[None, :]], 0)
    L = np.concatenate([Zt, np.ones((1, NM), np.float32)], 0)
    zlr[:, 0:512] = R[:, 0:512]
    zlr[:, 512:640] = L[:, 0:128]
    zlr[:, 640:896] = R[:, 512:768]
    zlr[:, 896:] = L[:, 128:768]

    idm = np.tile(np.eye(64, dtype=np.float32), (2, 1))

    maps = []
    for cid in range(NC):
        ka, kb = (0, 1) if cid < 4 else (2, 3)
        q = cid % 4
        pm = perms[q * PPC:(q + 1) * PPC]

        A = np.zeros((ROWS, NM), np.float32)
        A[np.arange(PPC)[:, None], pm[:, :N]] = 1
        A[PPC, :N] = 1
        A[PPC + 1, N:] = 1
        astk = np.zeros((128, NM), np.float32)
        astk[0:ROWS] = A
        astk[64:64 + ROWS] = A
        atp = np.zeros((128, 6 * D), np.float32)
        for c in range(6):
            atp[:, D * c:D * c + ROWS] = A[:, 128 * c:128 * (c + 1)].T
        A1 = A[:PPC, :N]
        A2 = A[:PPC, N:]
        Wc = (-KAP * (A1 * A2) + CB1 * A1 + CB2 * A2).astype(np.float32)
        wct = np.zeros((128, 3 * D), np.float32)
        for c in range(3):
            wct[:, D * c:D * c + PPC] = Wc[:, 128 * c:128 * (c + 1)].T
        bfp = np.zeros((128, BFP_W), np.float32)
        bfp[:, 0:6 * D] = atp
        bfp[:, 6 * D:9 * D] = wct
        bfp[:, 9 * D:9 * D + NM] = astk

        # pair d2 (host-reduced, f64): perm p pair j at lane (384p+j)%128,
        # block (384p+j)//128. Stripe pairs (j, 384+j) fill blocks
        # 3*PPC..3*PPC+2; stripe hits inside perm rows get a huge sentinel
        # so exp -> 0 (the zeroed K stripe).
        pX = pm[:, :N].astype(np.int64).ravel()
        pY = pm[:, N:].astype(np.int64).ravel()
        pd2 = ((Zf[pX].astype(np.float64) - Zf[pY]) ** 2).sum(1) + BIAS
        pd2[pY == pX + N] = 4e6
        sd2 = ((Zf[:N].astype(np.float64) - Zf[N:]) ** 2).sum(1) + BIAS
        pd2 = np.concatenate([pd2, sd2], 0)

        fsp = np.zeros((128, FSP_W), np.float32)
        fsp[:, 32:96] = idm
        fsp[:, 96:96 + NBLK] = pd2.reshape(NBLK, 128).T
        fsp[:, 96 + NBLK:FSP_W] = np.sqrt(pd2).reshape(NBLK, 128).T
        ga = np.float32(-1.0 / (b[ka] * b[ka]))
        lb = np.float32(-1.0 / b[kb])
        sqcols = sq.reshape(6, 128).T
        fsp[:, 0:6] = sqcols
        fsp[:, 12] = ga
        fsp[:, 13] = lb
        fsp[:, 14] = 0.0

        maps.append(dict(zlr=zlr, bfp=bfp.astype(ml_dtypes.bfloat16),
                         fsp=fsp))
    return maps


_NC_CACHE = None


def _get_nc():
    global _NC_CACHE
    if _NC_CACHE is None:
        _NC_CACHE = _build()
    return _NC_CACHE


def _merge(results, bandwidths):
    b = np.asarray(bandwidths, np.float64)
    full = np.zeros((4, 1 + NPER), np.float32)
    for cid in range(NC):
        ka, kb = (0, 1) if cid < 4 else (2, 3)
        q = cid % 4
        o = results[cid]["out"].astype(np.float64).ravel()
        d0a = np.exp(-BIAS / (b[ka] * b[ka]))
        d0b = np.exp(-np.sqrt(BIAS) / b[kb])
        for i, (k, d0) in enumerate(((kb, d0b), (ka, d0a))):
            base = 64 * i
            ubv = o[base:base + PPC]
            XXv = o[128 + base + PPC]
            YYv = o[128 + base + PPC + 1]
            aXv = o[256 + base + PPC]
            aYv = o[256 + base + PPC + 1]
            sev = o[384 + base + PPC]
            aux = NM * d0 * float(IC1)
            ck = (aXv + aYv - sev) * float(IC1) - aux
            u1 = (XXv + YYv) * float(IC1) - aux
            u2 = aXv - XXv - sev
            U = u2 * (-2.0 * float(IC2)) + u1
            full[k, 1 + q * PPC:1 + (q + 1) * PPC] = ubv + ck
            if q == 0:
                full[k, 0] = U
    return full


def kernel(X, Y, bandwidths, perms):
    nc = _get_nc()
    in_maps = _host_prep(X, Y, bandwidths, perms)
    res = bass_utils.run_bass_kernel_spmd(nc, in_maps, list(range(NC)))
    return _merge(res.results, bandwidths)


# revision 21
# speedup vs baseline: 1.0296x; 1.0296x over previous
"""Trainium2 Bass kernel for the 4-kernel MMD permutation test (nn_DUAL_78237124264373).

Sharding: 8 cores = 2 kernel-pairs x 4 permutation quarters. Core c<4 computes
kernels (0,1) for perms [50*(c%4), 50*(c%4)+50); core c>=4 the same for
kernels (2,3). Each core evaluates TWO kernel matrices, LAPLACIAN FIRST:
the SQRT activation table is resident from the start (warm op), dist is
computed straight out of the d2 PSUM tiles with the per-row-tile sq column
fused as the activation bias, and there is a single table swap to EXP that
serves both the laplacian and the gaussian passes. Order across the swap is
pinned with zero-valued bias/scale tokens that data-depend on the previous
block's last op (baseline trick), so the Tile scheduler cannot interleave.

Per-core pipeline (slot a = laplacian, slot b = gaussian):
  d2 = L^T R on the PE in f32r (L = [Zt; 1], R = [-2 Zt; sq + B]), 12 PSUM
  pieces (6 row tiles x 512+256) in a 4-deep PSUM pool, inputs spread over
  all five DMA queue rings so the first matmul is DMA-gated at ~9us instead
  of ~11us. Scalar: sqrt(ps + sq) -> dist per tile trailing the matmuls,
  then ONE swap to EXP; slot-a K = exp(lb*dist) chunk-by-chunk with its
  M0 = A_aug K matmuls right behind, then slot-b K = exp(ga*d2sb) the same
  way (d2sb staged by the DVE from PSUM during the d2 phase). Each slot's
  row stats (aKa, aK1, colA), and the PE transpose into the partition-0
  assembly row run as soon as its M0 completes; slot-b's aK1 rowsum rides
  the idle Pool engine in parallel with the DVE. The final affine assembly
  (U, ck, U_b = ubv + ck) happens on the host from the DMA'd 512-wide
  stats row - it is a per-core scalar epilogue, part of the gather.
"""

import sys

import numpy as np

if "/opt/trn_rl_repo" not in sys.path:
    sys.path.insert(0, "/opt/trn_rl_repo")

import ml_dtypes

import concourse.bacc as bacc
import concourse.bass as bass
import concourse.mybir as mybir
import concourse.tile as tile
from concourse import bass_utils

N = 384
NM = 768
D = 64
NPER = 200
NC = 8
PPC = 50                      # perms per core
ROWS = PPC + 2                # + X-identity + Y-identity rows
NBLK = 3 * PPC + 3            # pair blocks of 128: 50 perms x 3 + stripe x 3
BIAS = 0.0625                 # keeps d2 > 0 under f32r rounding (sqrt input)
C1 = float(N * (N - 1))
C2 = float(N * N)
KAP = np.float32(2.0 / C1 + 2.0 / C2)
CB1 = np.float32(1.0 / C1 + 2.0 / C2)
CB2 = np.float32(1.0 / C1)
TCO = np.float32(2.0 / C2)
IC1 = np.float32(1.0 / C1)
IC2 = np.float32(1.0 / C2)

F32 = mybir.dt.float32
F32R = mybir.dt.float32r
BF16 = mybir.dt.bfloat16
AF = mybir.ActivationFunctionType
ALU = mybir.AluOpType

BFP_W = 6 * D + 3 * D + NM           # atp | wct | astk (need-ordered)
FSP_W = 96 + 2 * NBLK                # consts | prd2 | prds


def _build():
    nc = bacc.Bacc("TRN2", target_bir_lowering=False, debug=False)
    with tile.TileContext(nc) as tc:
        with tc.tile_pool(name="dram", bufs=1, space="DRAM") as dram, \
             tc.tile_pool(name="io", bufs=1) as io, \
             tc.tile_pool(name="big", bufs=1) as big, \
             tc.tile_pool(name="scr", bufs=1) as scr, \
             tc.tile_pool(name="sml", bufs=1) as sml:

            def din(name, shape, dt=F32):
                return dram.tile(shape, dt, kind="ExternalInput", name=name,
                                 uniquify=False)

            zlr_d = din("zlr", [D + 1, 2 * NM], F32R)
            bfp_d = din("bfp", [128, BFP_W], BF16)
            fsp_d = din("fsp", [128, FSP_W], F32)
            out_d = dram.tile([1, 512], F32, kind="ExternalOutput",
                              name="out", uniquify=False)

            # ---- input DMAs spread over the 3 DMA-capable queue rings
            # (~45GB/s each); first-needed pieces lead their ring ----
            fsp = io.tile([128, FSP_W], F32, name="fsp_sb")
            zlr = io.tile([D + 1, 2 * NM], F32R, name="zlr_sb")
            bfp = io.tile([128, BFP_W], BF16, name="bfp_sb")
            nc.scalar.dma_start(out=fsp[:, 0:96], in_=fsp_d[:, 0:96])
            nc.gpsimd.dma_start(out=zlr[:, 512:896], in_=zlr_d[:, 512:896])
            nc.sync.dma_start(out=zlr[:, 0:256], in_=zlr_d[:, 0:256])
            nc.scalar.dma_start(out=zlr[:, 256:512], in_=zlr_d[:, 256:512])
            nc.sync.dma_start(out=zlr[:, 896:1216], in_=zlr_d[:, 896:1216])
            nc.gpsimd.dma_start(out=bfp[:, 0:9 * D], in_=bfp_d[:, 0:9 * D])
            nc.sync.dma_start(out=zlr[:, 1216:1536],
                              in_=zlr_d[:, 1216:1536])
            # warm the SQRT activation table BEFORE the scalar queue's last
            # (big, late-needed) DMA issue so the table is up by the first
            # PSUM tile
            warm = sml.tile([128, 1], F32, name="warm")
            wone = io.tile([1, 1], F32, name="wone_sb")
            nc.vector.memset(wone[:], 1.0)
            nc.scalar.activation(warm[0:1, :], wone[:], AF.Sqrt,
                                 bias=0.0, scale=1.0)
            nc.scalar.dma_start(out=fsp[:, 96:FSP_W], in_=fsp_d[:, 96:FSP_W])
            nc.gpsimd.dma_start(out=bfp[:, 9 * D:BFP_W],
                                in_=bfp_d[:, 9 * D:BFP_W])

            zr1 = zlr[:, 0:512]
            zl0 = zlr[:, 512:640]
            zr2 = zlr[:, 640:896]
            zlrest = zlr[:, 896:2 * NM]
            idm = fsp[:, 32:96]
            atp = bfp[:, 0:6 * D]                    # A^T chunks, 64-padded
            wct = bfp[:, 6 * D:9 * D]
            astk = bfp[:, 9 * D:9 * D + NM]          # A rows at 0-51 / 64-115
            prd2 = fsp[:, 96:96 + NBLK]              # pair d2 (host-reduced)
            prds = fsp[:, 96 + NBLK:FSP_W]           # pair dist
            sqc = fsp[:, 0:6]                        # sq columns per row tile
            ga = fsp[:, 12:13]
            lb = fsp[:, 13:14]
            zero = fsp[:, 14:15]

            ones = io.tile([128, 1], F32, name="ones_sb")
            nc.vector.memset(ones[:], 1.0)
            onesb = io.tile([128, 1], BF16, name="onesb_sb")
            nc.vector.memset(onesb[:], 1.0)

            d2sb = big.tile([128, 6 * NM], F32, name="d2sb")
            dist = big.tile([128, 6 * NM], F32, name="dist_sb")
            kta = big.tile([128, 6 * NM], BF16, name="kta")
            ktb = big.tile([128, 6 * NM], BF16, name="ktb")
            frow = sml.tile([1, 512], F32, name="frow")

            # ---- d2 phase: f32r matmuls, one [128,768] PSUM tile per row
            # tile, 4 deep; sqrt(ps + sq) straight from PSUM -> dist;
            # DVE lands clamped d2 in SBUF for the gaussian exp ----
            with tc.tile_pool(name="psA", bufs=4, space="PSUM") as psA:
                for r in range(6):
                    lhs = (zl0 if r == 0 else
                           zlrest[:, 128 * (r - 1):128 * r])
                    ps_d2 = psA.tile([128, NM], F32, tag="d2",
                                     name=f"ps_d2_{r}")
                    nc.tensor.matmul(ps_d2[:, 0:512], lhs, zr1[:],
                                     start=True, stop=True)
                    nc.tensor.matmul(ps_d2[:, 512:NM], lhs, zr2[:],
                                     start=True, stop=True)
                    sl = slice(NM * r, NM * (r + 1))
                    # PSUM-direct sqrt: BIAS = 1/16 dominates the f32r
                    # rounding of the diagonal (d2_true = BIAS) by ~15x
                    nc.scalar.activation(dist[:, sl], ps_d2[:], AF.Sqrt,
                                         scale=1.0, bias=sqc[:, r:r + 1])
                    nc.vector.tensor_scalar(
                        out=d2sb[:, sl], in0=ps_d2[:],
                        scalar1=sqc[:, r:r + 1], scalar2=0.0,
                        op0=ALU.add, op1=ALU.max)

            with tc.tile_pool(name="psB", bufs=1, space="PSUM") as psB, \
                 tc.tile_pool(name="psC", bufs=1, space="PSUM") as psC:

                ps_m = psB.tile([128, NM], F32, name="ps_m")
                ps_tc = psC.tile([128, 2], F32, name="ps_tc")
                ps_t = ps_tc[:, 0:1]
                ps_corr = ps_tc[:, 1:2]
                ps_row = ps_m[0:1, 0:512]

                # lbt == lb, but depends on the last column of EVERY sqrt
                # tile, so no SQRT op can be scheduled after the EXP swap
                dl0 = dist[:, NM - 1:NM]
                dlast = bass.AP(dl0.tensor, dl0.offset, [dl0.ap[0], [NM, 6]])
                zt6 = sml.tile([128, 1], F32, name="zt6")
                nc.vector.tensor_reduce(zt6[:], dlast,
                                        axis=mybir.AxisListType.X,
                                        op=ALU.max)
                lbt = sml.tile([128, 1], F32, name="lbt")
                nc.gpsimd.tensor_scalar(
                    out=lbt[:], in0=zt6[:], scalar1=0.0,
                    scalar2=lb, op0=ALU.mult, op1=ALU.add)

                def slot_tail(i, pe):
                    """Pair sums, corrections, row stats and the PE
                    transpose for slot i; runs as soon as its M0 stops.
                    All tiles are slot-private so the two slots' stats
                    never serialize on false cross-engine tile deps."""
                    pt = slice(64 * i, 64 * i + 64)
                    pack = sml.tile([128, 4], F32, name=f"pack{i}")
                    acb = sml.tile([128, 1], F32, name=f"acb{i}")
                    sB = scr.tile([128, NM], F32, name=f"sB{i}")
                    M0sb = scr.tile([128, NM], F32, name=f"M0sb{i}")
                    arow_ap = pack[pt, 2:3] if i == 0 else acb[pt, 0:1]
                    # t3: per-perm 3-block partial sums; group PPC holds the
                    # stripe so t[50] = sum(e) lands in ps_t for free
                    pe3 = pe.rearrange("p (g t) -> p g t", t=3)
                    t3 = sml.tile([128, 64], BF16, name=f"t3_{i}")
                    nc.vector.memset(t3[:, PPC + 1:64], 0.0)
                    with nc.allow_low_precision(reason="3-wide bf16 sum"):
                        nc.vector.tensor_reduce(t3[:, 0:PPC + 1], pe3[:],
                                                axis=mybir.AxisListType.X,
                                                op=ALU.add)
                    nc.tensor.matmul(ps_t[pt, :], t3[:], onesb[:],
                                     start=True, stop=True,
                                     tile_position=(0, 64 * i),
                                     skip_group_check=True)
                    for c in range(3):
                        nc.tensor.matmul(
                            ps_corr[pt, :], wct[:, D * c:D * c + 64],
                            pe[:, 3 * PPC + c:3 * PPC + c + 1],
                            start=(c == 0), stop=(c == 2),
                            tile_position=(0, 64 * i),
                            skip_group_check=True)
                    # row stats off this slot's half of ps_m; slot b's aK1
                    # rowsum rides the idle Pool engine in parallel
                    nc.vector.scalar_tensor_tensor(
                        out=sB[pt, :], in0=ps_m[pt, :], scalar=1.0,
                        in1=astk[pt, :], op0=ALU.mult, op1=ALU.mult,
                        accum_out=pack[pt, 1:2])
                    if i == 1:
                        nc.scalar.activation(M0sb[pt, :], ps_m[pt, :],
                                             AF.Copy, bias=0.0, scale=1.0,
                                             accum_out=arow_ap)
                    else:
                        nc.vector.tensor_scalar(
                            out=M0sb[pt, :], in0=ps_m[pt, :], scalar1=1.0,
                            scalar2=0.0, op0=ALU.mult, op1=ALU.add,
                            accum_out=arow_ap)
                    # ubv = KAP*(q0 - arow) + corr + TCO*t into pack col 0;
                    # q0 / arow / t stay in cols 1-3 for the transpose
                    nc.vector.tensor_tensor(out=pack[pt, 0:1],
                                            in0=pack[pt, 1:2],
                                            in1=arow_ap,
                                            op=ALU.subtract)
                    nc.vector.scalar_tensor_tensor(
                        out=pack[pt, 0:1], in0=pack[pt, 0:1],
                        scalar=float(KAP), in1=ps_corr[pt, :],
                        op0=ALU.mult, op1=ALU.add)
                    nc.vector.scalar_tensor_tensor(
                        out=pack[pt, 0:1], in0=ps_t[pt, :],
                        scalar=float(TCO), in1=pack[pt, 0:1],
                        op0=ALU.mult, op1=ALU.add)
                    nc.vector.tensor_copy(pack[pt, 3:4], ps_t[pt, :])
                    # transpose the 4 pack columns into the partition-0 row
                    # (ps_m bank 0 is free again: stats above read it first)
                    for k in range(4):
                        src = arow_ap if k == 2 else pack[pt, k:k + 1]
                        nc.tensor.matmul(
                            ps_row[0:1,
                                   128 * k + 64 * i:128 * k + 64 * i + 64],
                            src, idm[pt, :],
                            is_transpose=True, start=True, stop=True,
                            tile_position=(64 * i, 0),
                            skip_group_check=True)
                    s0 = ps_row[0:1, 64 * i:64 * i + 1]
                    f0 = frow[0:1, 64 * i:64 * i + 1]
                    nc.vector.tensor_copy(
                        bass.AP(f0.tensor, f0.offset,
                                [f0.ap[0], [128, 4], [1, 64]]),
                        bass.AP(s0.tensor, s0.offset,
                                [s0.ap[0], [128, 4], [1, 64]]))

                # ---- slot a (laplacian): one swap to EXP, K chunks with
                # M0 right behind; pair exp after the first chunk ----
                pel = sml.tile([128, NBLK], BF16, name="pel")
                for c in range(3):
                    cs = slice(2 * NM * c, 2 * NM * (c + 1))
                    nc.scalar.activation(ktb[:, cs], dist[:, cs], AF.Exp,
                                         scale=lbt, bias=zero)
                    if c == 0:
                        nc.scalar.activation(pel[:], prds[:], AF.Exp,
                                             bias=zero, scale=lbt)
                    for r in (2 * c, 2 * c + 1):
                        for fs in (slice(0, 512), slice(512, NM)):
                            nc.tensor.matmul(ps_m[0:64, fs],
                                             atp[:, D * r:D * r + 64],
                                             ktb[:, NM * r + fs.start:
                                                  NM * r + fs.stop],
                                             start=(r == 0), stop=(r == 5),
                                             tile_position=(0, 0),
                                             skip_group_check=True)
                slot_tail(0, pel)

                # zbg: zero bias that depends on the last slot-a EXP op so
                # the gaussian block cannot be scheduled before it
                zbg1 = sml.tile([128, 1], F32, name="zbg1")
                nc.gpsimd.tensor_scalar(
                    out=zbg1[:], in0=ktb[:, 6 * NM - 1:6 * NM],
                    scalar1=0.0, scalar2=0.0, op0=ALU.mult, op1=ALU.add)
                zbg = sml.tile([128, 1], F32, name="zbg")
                nc.gpsimd.tensor_tensor(out=zbg[:], in0=zbg1[:],
                                        in1=pel[:, 0:1], op=ALU.mult)
                # zbg-gated copy of atp pins the slot-b M0 ordering on the
                # in-order PE queue behind the slot-a block
                atp2 = scr.tile([128, 6 * D], BF16, name="atp2")
                nc.gpsimd.tensor_scalar(
                    out=atp2[:], in0=atp[:], scalar1=1.0, scalar2=zbg[:],
                    op0=ALU.mult, op1=ALU.add)

                # ---- slot b (gaussian): exp(ga*d2sb) chunks + pair exp ----
                peg = sml.tile([128, NBLK], BF16, name="peg")
                for c in range(3):
                    cs = slice(2 * NM * c, 2 * NM * (c + 1))
                    nc.scalar.activation(kta[:, cs], d2sb[:, cs], AF.Exp,
                                         scale=ga, bias=zbg)
                    if c == 0:
                        nc.scalar.activation(peg[:], prd2[:], AF.Exp,
                                             bias=zbg, scale=ga)
                    for r in (2 * c, 2 * c + 1):
                        for fs in (slice(0, 512), slice(512, NM)):
                            nc.tensor.matmul(ps_m[64:128, fs],
                                             atp2[:, D * r:D * r + 64],
                                             kta[:, NM * r + fs.start:
                                                  NM * r + fs.stop],
                                             start=(r == 0), stop=(r == 5),
                                             tile_position=(0, 64),
                                             skip_group_check=True)
                slot_tail(1, peg)

                # raw stats row out; the affine U/ck/U_b assembly is host-side
                nc.sync.dma_start(out=out_d[:, :], in_=frow[0:1, :])

    nc.compile()
    return nc


def _host_prep(X, Y, bandwidths, perms):
    X = np.ascontiguousarray(X, np.float32)
    Y = np.ascontiguousarray(Y, np.float32)
    perms = np.ascontiguousarray(perms, np.int32)
    Zf = np.concatenate([X, Y], 0)
    Zt = Zf.T.astype(np.float32)
    sq = (Zf.astype(np.float64) ** 2).sum(1).astype(np.float32)
    b = np.asarray(bandwidths, np.float64)

    zlr = np.zeros((D + 1, 2 * NM), np.float32)
    R = np.concatenate([-2.0 * Zt, (sq + BIAS)[None, :]], 0)
    L = np.concatenate([Zt, np.ones((1, NM), np.float32)], 0)
    zlr[:, 0:512] = R[:, 0:512]
    zlr[:, 512:640] = L[:, 0:128]
    zlr[:, 640:896] = R[:, 512:768]
    zlr[:, 896:] = L[:, 128:768]

    idm = np.tile(np.eye(64, dtype=np.float32), (2, 1))

    maps = []
    for cid in range(NC):
        ka, kb = (0, 1) if cid < 4 else (2, 3)
        q = cid % 4
        pm = perms[q * PPC:(q + 1) * PPC]

        A = np.zeros((ROWS, NM), np.float32)
        A[np.arange(PPC)[:, None], pm[:, :N]] = 1
        A[PPC, :N] = 1
        A[PPC + 1, N:] = 1
        astk = np.zeros((128, NM), np.float32)
        astk[0:ROWS] = A
        astk[64:64 + ROWS] = A
        atp = np.zeros((128, 6 * D), np.float32)
        for c in range(6):
            atp[:, D * c:D * c + ROWS] = A[:, 128 * c:128 * (c + 1)].T
        A1 = A[:PPC, :N]
        A2 = A[:PPC, N:]
        Wc = (-KAP * (A1 * A2) + CB1 * A1 + CB2 * A2).astype(np.float32)
        wct = np.zeros((128, 3 * D), np.float32)
        for c in range(3):
            wct[:, D * c:D * c + PPC] = Wc[:, 128 * c:128 * (c + 1)].T
        bfp = np.zeros((128, BFP_W), np.float32)
        bfp[:, 0:6 * D] = atp
        bfp[:, 6 * D:9 * D] = wct
        bfp[:, 9 * D:9 * D + NM] = astk

        # pair d2 (host-reduced, f64): perm p pair j at lane (384p+j)%128,
        # block (384p+j)//128. Stripe pairs (j, 384+j) fill blocks
        # 3*PPC..3*PPC+2; stripe hits inside perm rows get a huge sentinel
        # so exp -> 0 (the zeroed K stripe).
        pX = pm[:, :N].astype(np.int64).ravel()
        pY = pm[:, N:].astype(np.int64).ravel()
        pd2 = ((Zf[pX].astype(np.float64) - Zf[pY]) ** 2).sum(1) + BIAS
        pd2[pY == pX + N] = 4e6
        sd2 = ((Zf[:N].astype(np.float64) - Zf[N:]) ** 2).sum(1) + BIAS
        pd2 = np.concatenate([pd2, sd2], 0)

        fsp = np.zeros((128, FSP_W), np.float32)
        fsp[:, 32:96] = idm
        fsp[:, 96:96 + NBLK] = pd2.reshape(NBLK, 128).T
        fsp[:, 96 + NBLK:FSP_W] = np.sqrt(pd2).reshape(NBLK, 128).T
        ga = np.float32(-1.0 / (b[ka] * b[ka]))
        lb = np.float32(-1.0 / b[kb])
        sqcols = sq.reshape(6, 128).T
        fsp[:, 0:6] = sqcols
        fsp[:, 12] = ga
        fsp[:, 13] = lb
        fsp[:, 14] = 0.0

        maps.append(dict(zlr=zlr, bfp=bfp.astype(ml_dtypes.bfloat16),
                         fsp=fsp))
    return maps


_NC_CACHE = None


def _get_nc():
    global _NC_CACHE
    if _NC_CACHE is None:
        _NC_CACHE = _build()
    return _NC_CACHE


def _merge(results, bandwidths):
    b = np.asarray(bandwidths, np.float64)
    full = np.zeros((4, 1 + NPER), np.float32)
    for cid in range(NC):
        ka, kb = (0, 1) if cid < 4 else (2, 3)
        q = cid % 4
        o = results[cid]["out"].astype(np.float64).ravel()
        d0a = np.exp(-BIAS / (b[ka] * b[ka]))
        d0b = np.exp(-np.sqrt(BIAS) / b[kb])
        for i, (k, d0) in enumerate(((kb, d0b), (ka, d0a))):
            base = 64 * i
            ubv = o[base:base + PPC]
            XXv = o[128 + base + PPC]
            YYv = o[128 + base + PPC + 1]
            aXv = o[256 + base + PPC]
            aYv = o[256 + base + PPC + 1]
            sev = o[384 + base + PPC]
            aux = NM * d0 * float(IC1)
            ck = (aXv + aYv - sev) * float(IC1) - aux
            u1 = (XXv + YYv) * float(IC1) - aux
            u2 = aXv - XXv - sev
            U = u2 * (-2.0 * float(IC2)) + u1
            full[k, 1 + q * PPC:1 + (q + 1) * PPC] = ubv + ck
            if q == 0:
                full[k, 0] = U
    return full


def kernel(X, Y, bandwidths, perms):
    nc = _get_nc()
    in_maps = _host_prep(X, Y, bandwidths, perms)
    res = bass_utils.run_bass_kernel_spmd(nc, in_maps, list(range(NC)))
    return _merge(res.results, bandwidths)
